# revision 27
# baseline (speedup 1.0000x reference)
"""Trainium2 Bass kernel for batched weighted scatter-add (AttentionCopy).

Computes out[b, o, v] = sum_i attn[b, o, i] * (ids[b, i] == v)
for ids [16, 512] int32 in [0, 50000), attn [16, 32, 512] f32,
out [16, 32, 50000] f32.

Strategy: pure data parallel over batch (2 batches per core on 8 cores).
The output is 99% zeros (<=512 of 50000 columns are non-zero per batch), so
instead of dense one-hot matmuls (PE-bound at ~50us), the kernel:

  1. Zero-fills the whole per-core output (12.8 MB) with large coalesced
     DMAs from an SBUF zeros tile -- this runs at the HBM write roofline
     and is the unavoidable cost of materializing the dense output.  The
     DMA partition dim must be 128 (the hardware sprays descriptors over
     engines by evenly dividing the partition count; 125 partitions would
     use only 5 of 16 engines).
  2. Resolves duplicate ids with the selection-matrix trick: C[j, s] =
     (ids_j == slot_s) built by DVE compares, then ST = C.T @ attnT on the
     PE (tiny matmuls) so every slot holds the full collision sum for its
     column.  Duplicate slots hold identical rows, making duplicate
     scatter writes benign (plain overwrite, no read-modify-write).
  3. Scatters the non-zero columns with indirect DMAs (one index per
     partition, each writing a contiguous 32-float row of the v-major
     bucket tensor).

The output is split into one DRAM tensor per (batch, vocab bucket).  The
tile framework tracks DRAM write-write hazards per tensor, so each
bucket's scatter automatically waits only for that bucket's zero-fill
DMA, and all zero-fill/scatter pipelines overlap.  Columns are bucketed
into their bucket's 128 index slots on the host (pure index
preprocessing); empty slots are padded with a duplicate of a real column
in the same bucket (identical payload -> benign) or, for an empty
bucket, with id -1 (all-zero payload written to a row that has no real
column).  NOCT=6 buckets per batch keeps bucket occupancy <= 128 with
margin for the benchmark's id distribution; if an input ever overflows,
the kernel transparently recompiles with NOCT=8.

The device output is v-major; the host unshard step reassembles buckets
and transposes each batch to the required [32, 50000] row-major layout.
"""

import sys

sys.path.insert(0, "/opt/trn_rl_repo")

import numpy as np

NCORES = 8
B, O, I = 16, 32, 512
SIZE = 50000
BPC = B // NCORES  # batches per core
NCHUNK = I // 128  # 4 contraction chunks of 128
NOCT = 6  # vocab buckets per batch (fallback: 8 if a bucket overflows)


def _geom(noct):
    # bucket row span; rows % 4 == 0 so rows*O is 128-partition divisible
    step = (SIZE // noct // 4) * 4
    sizes = [step] * (noct - 1) + [SIZE - (noct - 1) * step]
    return step, sizes


_cache = {}


def _build(noct):
    import concourse.bacc as bacc
    import concourse.bass as bass
    import concourse.mybir as mybir
    import concourse.tile as tile

    f32 = mybir.dt.float32
    bf16 = mybir.dt.bfloat16
    i32 = mybir.dt.int32
    u16 = mybir.dt.uint16
    Alu = mybir.AluOpType

    ostep, osizes = _geom(noct)
    slots = noct * 128  # column slots per batch (128 per bucket)

    nc = bacc.Bacc("TRN2", target_bir_lowering=False, debug=False, num_devices=NCORES)

    # slot column ids (u16 -- ids < 50000 < 65536), replicated on all
    # partitions: idsb[p, b*slots + q*128 + s] = id of slot (b, q, s)
    idsb_d = nc.dram_tensor("idsb", [128, BPC * slots], u16, kind="ExternalInput").ap()
    # per-partition contraction ids: idspp[p, b*4 + c] = ids[b, c*128 + p]
    idspp_d = nc.dram_tensor("idspp", [128, BPC * NCHUNK], f32, kind="ExternalInput").ap()
    # bucket-local scatter rows: idx[p, b*noct + q] = slot (b,q,p)'s local row
    idx_d = nc.dram_tensor("idx", [128, BPC * noct], i32, kind="ExternalInput").ap()
    # attn pre-tiled on host: attn[p, (b, c, o)] = attn[b, o, c*128 + p]
    attn_d = nc.dram_tensor(
        "attn", [128, BPC * NCHUNK * O], f32, kind="ExternalInput"
    ).ap()
    # v-major outputs, one tensor per (batch, bucket):
    # out_b{b}q{q}[r, o] = out[b, o, q*ostep + r]
    out_d = [
        [
            nc.dram_tensor(f"out_b{b}q{q}", [osizes[q], O], f32, kind="ExternalOutput").ap()
            for q in range(noct)
        ]
        for b in range(BPC)
    ]

    with tile.TileContext(nc) as tc:
        with (
            tc.tile_pool(name="zeros", bufs=1) as zp,
            tc.tile_pool(name="inp", bufs=1) as inp,
            tc.tile_pool(name="work", bufs=1) as wp,
            tc.tile_pool(name="psst", bufs=4, space="PSUM") as psp,
        ):
            # --- zeros tile first: its memset gates the zero-fill stream;
            # split across gpsimd and vector so it finishes in half the time
            ztc = max(osizes) * O // 128
            zt = zp.tile([128, ztc], f32)
            nc.gpsimd.memset(zt[:, 0 : ztc // 2], 0)
            nc.vector.memset(zt[:, ztc // 2 : ztc], 0)

            # --- inputs on the scalar engine, biggest/most-gating first;
            # the sync engine mostly dispatches zero-fill (those dispatches
            # block on completion-lane chains) so compute is never stuck
            # behind the zero-fill and vice versa.
            idsb = inp.tile([128, BPC * slots], u16)
            nc.scalar.dma_start(out=idsb[:], in_=idsb_d[:])
            at_f = inp.tile([128, BPC * NCHUNK * O], f32)
            nc.scalar.dma_start(out=at_f[:], in_=attn_d[:])
            idspp = inp.tile([128, BPC * NCHUNK], f32)
            nc.scalar.dma_start(out=idspp[:], in_=idspp_d[:])
            idx = inp.tile([128, BPC * noct], i32)
            nc.scalar.dma_start(out=idx[:], in_=idx_d[:])
            atb = inp.tile([128, BPC * NCHUNK * O], bf16)
            nc.vector.tensor_copy(out=atb[:], in_=at_f[:])

            # --- zero-fill DMAs (one per bucket tensor, 128 partitions
            # each so all 16 DMA engines are used; both HWDGE engines
            # dispatch so the stream ramps quickly)
            for b in range(BPC):
                for q in range(noct):
                    zc = osizes[q] * O // 128
                    k = b * noct + q
                    # every 3rd dispatch rides gpsimd's SWDGE path: extra
                    # DMA completion lanes beyond the 8 HWDGE ones, so more
                    # zero-fill DMAs stay in flight
                    eng = (nc.sync, nc.scalar, nc.gpsimd)[2 if k % 3 == 2 else k % 2]
                    eng.dma_start(
                        out=out_d[b][q]
                        .rearrange("r o -> (r o)")
                        .rearrange("(p f) -> p f", f=zc),
                        in_=zt[:, 0:zc],
                    )

            # --- C[j, s] = (ids_j == slot_s), bf16 0/1, per (batch, j-chunk)
            cmats = []
            for b in range(BPC):
                cmat = wp.tile([128, NCHUNK * slots], bf16, name=f"c{b}")
                cmats.append(cmat)
                for cj in range(NCHUNK):
                    nc.vector.tensor_scalar(
                        out=cmat[:, cj * slots : (cj + 1) * slots],
                        in0=idsb[:, b * slots : (b + 1) * slots],
                        scalar1=idspp[:, b * NCHUNK + cj : b * NCHUNK + cj + 1],
                        scalar2=None,
                        op0=Alu.is_equal,
                    )

            # --- per bucket: ST = C.T @ attnT (collision sums); PSUM
            # drained by the vector engine (scalar is busy dispatching the
            # zero-fill and would stall the copies behind it)
            valst = []
            for b in range(BPC):
                vals = wp.tile([128, noct * O], f32, name=f"v{b}")
                valst.append(vals)
                for q in range(noct):
                    pst = psp.tile([128, O], f32, tag="st")
                    for cj in range(NCHUNK):
                        nc.tensor.matmul(
                            out=pst[:],
                            lhsT=cmats[b][
                                :, cj * slots + q * 128 : cj * slots + (q + 1) * 128
                            ],
                            rhs=atb[:, (b * NCHUNK + cj) * O : (b * NCHUNK + cj + 1) * O],
                            start=(cj == 0),
                            stop=(cj == NCHUNK - 1),
                        )
                    nc.vector.tensor_copy(out=vals[:, q * O : (q + 1) * O], in_=pst[:])

            # --- indirect scatters: partition p writes vals[p, q*32:...] to
            # row idx[p, b*noct+q] of out_b{b}q{q}.  The DRAM WAW hazard on
            # the bucket tensor orders each one after that bucket's zero-fill.
            for b in range(BPC):
                for q in range(noct):
                    nc.gpsimd.indirect_dma_start(
                        out=out_d[b][q][:],
                        out_offset=bass.IndirectOffsetOnAxis(
                            ap=idx[:, b * noct + q : b * noct + q + 1], axis=0
                        ),
                        in_=valst[b][:, q * O : (q + 1) * O],
                        in_offset=None,
                    )

    nc.compile()
    return nc


def _bucketize(ids, noct):
    """Host-side index preprocessing.  Returns in_maps or None on overflow."""
    ostep, _ = _geom(noct)
    slots = noct * 128
    in_maps = []
    for core in range(NCORES):
        idsc = ids[core * BPC : (core + 1) * BPC]  # [BPC, I]
        # per-partition contraction ids (f32 exact below 2**24)
        pp = (
            idsc.astype(np.float32)
            .reshape(BPC, NCHUNK, 128)
            .transpose(2, 0, 1)
            .reshape(128, BPC * NCHUNK)
        )
        # bucket columns into (bucket, slot); pad 65535 never matches an id
        idsml = np.full((1, BPC * slots), 65535, dtype=np.uint16)
        idxt = np.zeros((128, BPC * noct), dtype=np.int32)
        for b in range(BPC):
            oct_of = np.minimum(idsc[b] // ostep, noct - 1)
            for q in range(noct):
                cols = idsc[b][oct_of == q]  # this bucket's column ids
                n = len(cols)
                if n > 128:
                    return None  # bucket overflow: caller retries wider
                if n:
                    slot = np.empty(128, dtype=np.int64)
                    slot[:n] = cols
                    slot[n:] = cols[0]  # duplicate pad: identical payload
                    idsml[0, (b * noct + q) * 128 : (b * noct + q + 1) * 128] = slot
                    idxt[:, b * noct + q] = slot - q * ostep
                # else: idsml stays 65535 (all-zero payload), idx stays 0
        in_maps.append(
            {
                "idsb": np.ascontiguousarray(
                    np.broadcast_to(idsml, (128, BPC * slots))
                ),
                "idspp": np.ascontiguousarray(pp),
                "idx": idxt,
                # [p, (b, c, o)] = attn[b, o, c*128 + p]
                "attn": None,
            }
        )
    return in_maps


def _in_maps(ids, attn, noct=None):
    ids = np.asarray(ids, dtype=np.int64)
    noct = noct if noct is not None else _pick_noct(ids)
    in_maps = _bucketize(ids, noct)
    assert in_maps is not None
    for core in range(NCORES):
        in_maps[core]["attn"] = np.ascontiguousarray(
            attn[core * BPC : (core + 1) * BPC]
            .reshape(BPC, O, NCHUNK, 128)
            .transpose(3, 0, 2, 1)
            .reshape(128, BPC * NCHUNK * O)
        )
    return in_maps


def _pick_noct(ids):
    ostep, _ = _geom(NOCT)
    for b in range(B):
        oct_of = np.minimum(ids[b] // ostep, NOCT - 1)
        if np.bincount(oct_of, minlength=NOCT).max() > 128:
            return 8
    return NOCT


def kernel(ids, attn):
    from concourse.bass_utils import run_bass_kernel_spmd

    ids = np.ascontiguousarray(ids, dtype=np.int32)
    attn = np.ascontiguousarray(attn, dtype=np.float32)

    noct = _pick_noct(np.asarray(ids, dtype=np.int64))
    if noct not in _cache:
        _cache[noct] = _build(noct)
    nc = _cache[noct]
    _cache["nc"] = nc  # for external profiling harnesses

    _, osizes = _geom(noct)
    core_ids = list(range(NCORES))
    res = run_bass_kernel_spmd(nc, _in_maps(ids, attn, noct), core_ids)
    # reassemble: per (core, batch) concat buckets -> [50000, 32] -> transpose
    out = np.empty((B, O, SIZE), dtype=np.float32)
    for c in core_ids:
        for b in range(BPC):
            vmaj = np.concatenate(
                [res.results[c][f"out_b{b}q{q}"] for q in range(len(osizes))], axis=0
            )
            out[c * BPC + b] = vmaj.T
    return out


# revision 28
# speedup vs baseline: 1.1528x; 1.1528x over previous
"""Trainium2 Bass kernel for batched weighted scatter-add (AttentionCopy).

Computes out[b, o, v] = sum_i attn[b, o, i] * (ids[b, i] == v)
for ids [16, 512] int32 in [0, 50000), attn [16, 32, 512] f32,
out [16, 32, 50000] f32.

Strategy: pure data parallel over batch (2 batches per core on 8 cores).
The output is 99% zeros (<=512 of 50000 columns are non-zero per batch), so
instead of dense one-hot matmuls (PE-bound at ~50us), the kernel:

  1. Zero-fills the whole per-core output (12.8 MB) with large coalesced
     DMAs from an SBUF zeros tile -- this runs at the HBM write roofline
     and is the unavoidable cost of materializing the dense output.  The
     DMA partition dim must be 128 (the hardware sprays descriptors over
     engines by evenly dividing the partition count; 125 partitions would
     use only 5 of 16 engines).
  2. Resolves duplicate ids with the selection-matrix trick: C[j, s] =
     (ids_j == slot_s) built by DVE compares, then ST = C.T @ attnT on the
     PE (tiny matmuls) so every slot holds the full collision sum for its
     column.  Duplicate slots hold identical rows, making duplicate
     scatter writes benign (plain overwrite, no read-modify-write).
  3. Scatters the non-zero columns with indirect DMAs (one index per
     partition, each writing a contiguous 32-float row of the v-major
     bucket tensor).

The output is split into one DRAM tensor per (batch, vocab bucket).  The
tile framework tracks DRAM write-write hazards per tensor, so each
bucket's scatter automatically waits only for that bucket's zero-fill
DMA, and all zero-fill/scatter pipelines overlap.  Columns are bucketed
into their bucket's 128 index slots on the host (pure index
preprocessing); empty slots are padded with a duplicate of a real column
in the same bucket (identical payload -> benign) or, for an empty
bucket, with id -1 (all-zero payload written to a row that has no real
column).  NOCT=6 buckets per batch keeps bucket occupancy <= 128 with
margin for the benchmark's id distribution; if an input ever overflows,
the kernel transparently recompiles with NOCT=8.

The device output is v-major; the host unshard step reassembles buckets
and transposes each batch to the required [32, 50000] row-major layout.
"""

import sys

sys.path.insert(0, "/opt/trn_rl_repo")

import numpy as np

NCORES = 8
B, O, I = 16, 32, 512
SIZE = 50000
BPC = B // NCORES  # batches per core
NCHUNK = I // 128  # 4 contraction chunks of 128
NOCT = 6  # vocab buckets per batch (fallback: 8 if a bucket overflows)


def _geom(noct):
    # bucket row span; rows % 4 == 0 so rows*O is 128-partition divisible
    step = (SIZE // noct // 4) * 4
    sizes = [step] * (noct - 1) + [SIZE - (noct - 1) * step]
    return step, sizes


_cache = {}


def _build(noct):
    import concourse.bacc as bacc
    import concourse.bass as bass
    import concourse.mybir as mybir
    import concourse.tile as tile

    f32 = mybir.dt.float32
    bf16 = mybir.dt.bfloat16
    i32 = mybir.dt.int32
    u16 = mybir.dt.uint16
    Alu = mybir.AluOpType

    ostep, osizes = _geom(noct)
    slots = noct * 128  # column slots per batch (128 per bucket)

    nc = bacc.Bacc("TRN2", target_bir_lowering=False, debug=False, num_devices=NCORES)

    # slot column ids (u16 -- ids < 50000 < 65536), replicated on all
    # partitions: idsb[p, b*slots + q*128 + s] = id of slot (b, q, s)
    idsb_d = nc.dram_tensor("idsb", [128, BPC * slots], u16, kind="ExternalInput").ap()
    # per-partition contraction ids: idspp[p, b*4 + c] = ids[b, c*128 + p]
    idspp_d = nc.dram_tensor("idspp", [128, BPC * NCHUNK], f32, kind="ExternalInput").ap()
    # bucket-local scatter rows: idx[p, b*noct + q] = slot (b,q,p)'s local row
    idx_d = nc.dram_tensor("idx", [128, BPC * noct], i32, kind="ExternalInput").ap()
    # attn pre-tiled on host: attn[p, (b, c, o)] = attn[b, o, c*128 + p]
    attn_d = nc.dram_tensor(
        "attn", [128, BPC * NCHUNK * O], f32, kind="ExternalInput"
    ).ap()
    # v-major outputs, one tensor per (batch, bucket):
    # out_b{b}q{q}[r, o] = out[b, o, q*ostep + r]
    out_d = [
        [
            nc.dram_tensor(f"out_b{b}q{q}", [osizes[q], O], f32, kind="ExternalOutput").ap()
            for q in range(noct)
        ]
        for b in range(BPC)
    ]

    with tile.TileContext(nc) as tc:
        with (
            tc.tile_pool(name="zeros", bufs=1) as zp,
            tc.tile_pool(name="inp", bufs=1) as inp,
            tc.tile_pool(name="work", bufs=1) as wp,
            tc.tile_pool(name="psst", bufs=4, space="PSUM") as psp,
        ):
            # --- zeros tile first: its memset gates the zero-fill stream;
            # split across gpsimd and vector so it finishes in half the time
            ztc = max(osizes) * O // 128
            zt = zp.tile([128, ztc], f32)
            nc.gpsimd.memset(zt[:, 0 : ztc // 2], 0)
            nc.vector.memset(zt[:, ztc // 2 : ztc], 0)

            # --- inputs on the scalar engine, biggest/most-gating first;
            # the sync engine mostly dispatches zero-fill (those dispatches
            # block on completion-lane chains) so compute is never stuck
            # behind the zero-fill and vice versa.
            idsb = inp.tile([128, BPC * slots], u16)
            nc.scalar.dma_start(out=idsb[:], in_=idsb_d[:])
            at_f = inp.tile([128, BPC * NCHUNK * O], f32)
            nc.scalar.dma_start(out=at_f[:], in_=attn_d[:])
            idspp = inp.tile([128, BPC * NCHUNK], f32)
            nc.scalar.dma_start(out=idspp[:], in_=idspp_d[:])
            idx = inp.tile([128, BPC * noct], i32)
            nc.scalar.dma_start(out=idx[:], in_=idx_d[:])
            atb = inp.tile([128, BPC * NCHUNK * O], bf16)
            nc.vector.tensor_copy(out=atb[:], in_=at_f[:])

            # --- zero-fill DMAs (one per bucket tensor, 128 partitions
            # each so all 16 DMA engines are used; both HWDGE engines
            # dispatch so the stream ramps quickly)
            for b in range(BPC):
                for q in range(noct):
                    zc = osizes[q] * O // 128
                    eng = (nc.sync, nc.scalar)[(b * noct + q) % 2]
                    eng.dma_start(
                        out=out_d[b][q]
                        .rearrange("r o -> (r o)")
                        .rearrange("(p f) -> p f", f=zc),
                        in_=zt[:, 0:zc],
                    )

            # --- C[j, s] = (ids_j == slot_s), bf16 0/1, per (batch, j-chunk)
            cmats = []
            for b in range(BPC):
                cmat = wp.tile([128, NCHUNK * slots], bf16, name=f"c{b}")
                cmats.append(cmat)
                for cj in range(NCHUNK):
                    nc.vector.tensor_scalar(
                        out=cmat[:, cj * slots : (cj + 1) * slots],
                        in0=idsb[:, b * slots : (b + 1) * slots],
                        scalar1=idspp[:, b * NCHUNK + cj : b * NCHUNK + cj + 1],
                        scalar2=None,
                        op0=Alu.is_equal,
                    )

            # --- per bucket: ST = C.T @ attnT (collision sums); PSUM
            # drained by the vector engine (scalar is busy dispatching the
            # zero-fill and would stall the copies behind it)
            valst = []
            for b in range(BPC):
                vals = wp.tile([128, noct * O], f32, name=f"v{b}")
                valst.append(vals)
                for q in range(noct):
                    pst = psp.tile([128, O], f32, tag="st")
                    for cj in range(NCHUNK):
                        nc.tensor.matmul(
                            out=pst[:],
                            lhsT=cmats[b][
                                :, cj * slots + q * 128 : cj * slots + (q + 1) * 128
                            ],
                            rhs=atb[:, (b * NCHUNK + cj) * O : (b * NCHUNK + cj + 1) * O],
                            start=(cj == 0),
                            stop=(cj == NCHUNK - 1),
                        )
                    nc.vector.tensor_copy(out=vals[:, q * O : (q + 1) * O], in_=pst[:])

            # --- indirect scatters: partition p writes vals[p, q*32:...] to
            # row idx[p, b*noct+q] of out_b{b}q{q}.  The DRAM WAW hazard on
            # the bucket tensor orders each one after that bucket's zero-fill.
            for b in range(BPC):
                for q in range(noct):
                    nc.gpsimd.indirect_dma_start(
                        out=out_d[b][q][:],
                        out_offset=bass.IndirectOffsetOnAxis(
                            ap=idx[:, b * noct + q : b * noct + q + 1], axis=0
                        ),
                        in_=valst[b][:, q * O : (q + 1) * O],
                        in_offset=None,
                    )

    nc.compile()
    return nc


def _bucketize(ids, noct):
    """Host-side index preprocessing.  Returns in_maps or None on overflow."""
    ostep, _ = _geom(noct)
    slots = noct * 128
    in_maps = []
    for core in range(NCORES):
        idsc = ids[core * BPC : (core + 1) * BPC]  # [BPC, I]
        # per-partition contraction ids (f32 exact below 2**24)
        pp = (
            idsc.astype(np.float32)
            .reshape(BPC, NCHUNK, 128)
            .transpose(2, 0, 1)
            .reshape(128, BPC * NCHUNK)
        )
        # bucket columns into (bucket, slot); pad 65535 never matches an id
        idsml = np.full((1, BPC * slots), 65535, dtype=np.uint16)
        idxt = np.zeros((128, BPC * noct), dtype=np.int32)
        for b in range(BPC):
            oct_of = np.minimum(idsc[b] // ostep, noct - 1)
            for q in range(noct):
                cols = idsc[b][oct_of == q]  # this bucket's column ids
                n = len(cols)
                if n > 128:
                    return None  # bucket overflow: caller retries wider
                if n:
                    slot = np.empty(128, dtype=np.int64)
                    slot[:n] = cols
                    slot[n:] = cols[0]  # duplicate pad: identical payload
                    idsml[0, (b * noct + q) * 128 : (b * noct + q + 1) * 128] = slot
                    idxt[:, b * noct + q] = slot - q * ostep
                # else: idsml stays 65535 (all-zero payload), idx stays 0
        in_maps.append(
            {
                "idsb": np.ascontiguousarray(
                    np.broadcast_to(idsml, (128, BPC * slots))
                ),
                "idspp": np.ascontiguousarray(pp),
                "idx": idxt,
                # [p, (b, c, o)] = attn[b, o, c*128 + p]
                "attn": None,
            }
        )
    return in_maps


def _in_maps(ids, attn, noct=None):
    ids = np.asarray(ids, dtype=np.int64)
    noct = noct if noct is not None else _pick_noct(ids)
    in_maps = _bucketize(ids, noct)
    assert in_maps is not None
    for core in range(NCORES):
        in_maps[core]["attn"] = np.ascontiguousarray(
            attn[core * BPC : (core + 1) * BPC]
            .reshape(BPC, O, NCHUNK, 128)
            .transpose(3, 0, 2, 1)
            .reshape(128, BPC * NCHUNK * O)
        )
    return in_maps


def _pick_noct(ids):
    ostep, _ = _geom(NOCT)
    for b in range(B):
        oct_of = np.minimum(ids[b] // ostep, NOCT - 1)
        if np.bincount(oct_of, minlength=NOCT).max() > 128:
            return 8
    return NOCT


def kernel(ids, attn):
    from concourse.bass_utils import run_bass_kernel_spmd

    ids = np.ascontiguousarray(ids, dtype=np.int32)
    attn = np.ascontiguousarray(attn, dtype=np.float32)

    noct = _pick_noct(np.asarray(ids, dtype=np.int64))
    if noct not in _cache:
        _cache[noct] = _build(noct)
    nc = _cache[noct]
    _cache["nc"] = nc  # for external profiling harnesses

    _, osizes = _geom(noct)
    core_ids = list(range(NCORES))
    res = run_bass_kernel_spmd(nc, _in_maps(ids, attn, noct), core_ids)
    # reassemble: per (core, batch) concat buckets -> [50000, 32] -> transpose
    out = np.empty((B, O, SIZE), dtype=np.float32)
    for c in core_ids:
        for b in range(BPC):
            vmaj = np.concatenate(
                [res.results[c][f"out_b{b}q{q}"] for q in range(len(osizes))], axis=0
            )
            out[c * BPC + b] = vmaj.T
    return out
